# revision 24
# baseline (speedup 1.0000x reference)
"""Log2Quantizer Trainium2 kernel (raw Bass, no Tile).

Math: the reference's sort/std/rank machinery is dead code (bit_token is
unconditionally overwritten with n_bits), so the computation reduces to:
    delta[b,t] = max over (h,c) of x[b,h,t,c]
    out = delta * 2^(round(log2(max(x/delta, 1e-8))))
i.e. snap x/delta to the nearest power of two in log space, rescale by delta.

Division-route bit-trick (no transcendentals), exact on the fp32-internal DVE:
    q  = x * (1 / (delta*sqrt2))             (reciprocal is IEEE 1/x on trn2)
    p2 = bitcast_f32(bits(q) & 0x7F800000)   # 2^floor(log2 q) = 2^(k-1)
    out = p2 * (2*delta)                     # fp32 mult by 2^k, exact
round(log2(x/delta)) = floor(log2(x/(delta*sqrt2))) + 1, so flooring q to its
exponent implements the rounding; x==0 gives q=0 -> p2=+0.0 -> out=0 (the
reference's 1e-8 ratio clamp yields delta*2^-27 ~ 7e-9 there; abs err 7e-9).

Sharding: data-parallel over batch dim b (8 rows -> 8 cores), no comms.
Layout: t split into chunks; partition dim = t-block so each partition line is
one contiguous run per h in DRAM. Per-token scalars broadcast along the free
(h, c) dims with stride-0 APs.

Engine split and pipeline (NBUF-deep buffers):
  DVE:    R1+R2 reduces, recip smalls, M1 mult, AND mask (~9us/chunk)
  GpSimd: M2 final fp32 mult (tensor_tensor)             (~5.3us/chunk)
  Sync:   HWDGE DMAs
Per-chunk chain: load -> R1,R2,smalls,M1,AND (dve) -> M2 (gp) -> store
Same-engine ops execute in order on trn2 (per-op pipeline drain), so only
cross-engine edges carry semaphores:
  dve_sem: +1 by AND (last DVE op of a chunk); gp waits it before M2
  v_sem:   +1 by M2 (gp); sync waits it before the store
  load_sem/store_sem[NBUF]: buffer-slot DMA completion sems (16/DMA); each
    slot's counter is only ever one-DMA-in-flight, so thresholds are exact
"""

from contextlib import ExitStack

import numpy as np

import concourse.bass as bass
import concourse.mybir as mybir
from concourse.bass_utils import run_bass_kernel_spmd

B, H, T, C = 8, 12, 4096, 64
N_CORES = 8
P = 128          # SBUF partitions
TC = 512         # tokens per chunk (pipeline granularity)
NBUF = 4         # xt/wt buffer depth

SQRT2 = 1.4142135623730951
EXP_MASK = 0x7F800000

_nc_cache = {}


def _build_nc():
    if "nc" in _nc_cache:
        return _nc_cache["nc"]
    f32 = mybir.dt.float32
    i32 = mybir.dt.int32
    OP = mybir.AluOpType
    X = mybir.AxisListType.X

    nc = bass.Bass()
    x_in = nc.declare_dram_parameter("x", [H, T, C], f32, isOutput=False)
    y_out = nc.declare_dram_parameter("y", [H, T, C], f32, isOutput=True)

    n_chunks = T // TC
    tt = TC // P
    FREE = H * tt * C

    def src_ap(ci):
        return x_in[:, ci * TC : (ci + 1) * TC, :].rearrange(
            "h (p q) c -> p h (q c)", p=P
        )

    def dst_ap(ci):
        return y_out[:, ci * TC : (ci + 1) * TC, :].rearrange(
            "h (p q) c -> p h (q c)", p=P
        )

    with ExitStack() as ctx:
        xt = [
            ctx.enter_context(nc.sbuf_tensor(f"xt{j}", [P, FREE], f32))
            for j in range(NBUF)
        ]
        wt = [
            ctx.enter_context(nc.sbuf_tensor(f"wt{j}", [P, FREE], f32))
            for j in range(NBUF)
        ]
        r1 = ctx.enter_context(nc.sbuf_tensor("r1", [P, H * tt], f32))
        delta = ctx.enter_context(nc.sbuf_tensor("delta", [P, tt], f32))
        tf = ctx.enter_context(nc.sbuf_tensor("tf", [P, tt], f32))
        inv = ctx.enter_context(nc.sbuf_tensor("inv", [P, tt], f32))
        # d2 is read cross-engine by gp's M2 -> per-slot copies so the next
        # DVE chunks never wait for M2
        d2 = [
            ctx.enter_context(nc.sbuf_tensor(f"d2_{j}", [P, tt], f32))
            for j in range(NBUF)
        ]

        load_sem = [
            ctx.enter_context(nc.semaphore(f"load_sem{j}")) for j in range(NBUF)
        ]
        store_sem = [
            ctx.enter_context(nc.semaphore(f"store_sem{j}")) for j in range(NBUF)
        ]
        v_sem = ctx.enter_context(nc.semaphore("v_sem"))
        dve_sem = ctx.enter_context(nc.semaphore("dve_sem"))

        block = ctx.enter_context(nc.Block())

        @block.sync
        def _(sync):
            for ci in range(n_chunks):
                sync.dma_start(out=xt[ci % NBUF][:], in_=src_ap(ci)).then_inc(
                    load_sem[ci % NBUF], 16
                )
                if ci >= 1:
                    sync.wait_ge(v_sem, ci)
                    sync.dma_start(
                        out=dst_ap(ci - 1), in_=wt[(ci - 1) % NBUF][:]
                    ).then_inc(store_sem[(ci - 1) % NBUF], 16)
            sync.wait_ge(v_sem, n_chunks)
            sync.dma_start(
                out=dst_ap(n_chunks - 1), in_=wt[(n_chunks - 1) % NBUF][:]
            ).then_inc(store_sem[(n_chunks - 1) % NBUF], 16)

        @block.gpsimd
        def _(gp):
            for ci in range(n_chunks):
                j = ci % NBUF
                xt4 = xt[j][:].rearrange("p (h q c) -> p h q c", h=H, c=C)
                wt4 = wt[j][:].rearrange("p (h q c) -> p h q c", h=H, c=C)

                # M2: out = p2 * 2*delta  (xt -> wt; wt dead after AND)
                d2_b = d2[j][:].unsqueeze(1).unsqueeze(3).broadcast_to([P, H, tt, C])
                gp.wait_ge(dve_sem, ci + 1)                     # AND(ci) done
                gp.tensor_tensor(out=wt4, in0=xt4, in1=d2_b, op=OP.mult).then_inc(
                    v_sem, 1
                )

        @block.vector
        def _(vector):
            for ci in range(n_chunks):
                j = ci % NBUF
                xt4 = xt[j][:].rearrange("p (h q c) -> p h q c", h=H, c=C)
                wt4 = wt[j][:].rearrange("p (h q c) -> p h q c", h=H, c=C)
                r13 = r1[:].rearrange("p (h q) -> p h q", h=H)

                if ci >= NBUF:
                    vector.wait_ge(store_sem[j], 16 * (ci // NBUF))  # wt free
                vector.wait_ge(load_sem[j], 16 * (ci // NBUF + 1))   # xt loaded

                # DVE ops overlap in the pipe unless drained; a DRAIN between
                # each dependent pair enforces same-engine RAW/WAR (this is
                # what Tile emits too). ~tens of ns each.
                # R1: per-(token,h) max over c; R2: delta = max over h
                vector.reduce_max(out=r13, in_=xt4, axis=X)
                vector.drain()
                vector.reduce_max(
                    out=delta[:], in_=r13.transpose([0, 2, 1]), axis=X
                )
                vector.drain()
                # per-token scalars
                vector.tensor_scalar_mul(tf[:], delta[:], SQRT2)
                vector.drain()
                vector.reciprocal(inv[:], tf[:])
                vector.tensor_scalar_mul(d2[j][:], delta[:], 2.0)
                vector.drain()

                inv_b = inv[:].unsqueeze(1).unsqueeze(3).broadcast_to([P, H, tt, C])

                # M1: q = x * inv
                vector.tensor_tensor(out=wt4, in0=xt4, in1=inv_b, op=OP.mult)
                vector.drain()
                # AND: p2 = bits(q) & 0x7F800000  (wt -> xt, xt dead after M1)
                vector.tensor_scalar(
                    out=xt[j][:].bitcast(i32),
                    in0=wt[j][:].bitcast(i32),
                    scalar1=EXP_MASK,
                    scalar2=None,
                    op0=OP.bitwise_and,
                ).then_inc(dve_sem, 1)

    _nc_cache["nc"] = nc
    return nc


def kernel(x: np.ndarray) -> np.ndarray:
    assert x.shape == (B, H, T, C) and x.dtype == np.float32
    nc = _build_nc()
    in_maps = [{"x": np.ascontiguousarray(x[i])} for i in range(N_CORES)]
    res = run_bass_kernel_spmd(nc, in_maps, list(range(N_CORES)))
    out = np.stack([res.results[i]["y"] for i in range(N_CORES)], axis=0)
    return out


# revision 26
# speedup vs baseline: 1.0632x; 1.0632x over previous
"""Log2Quantizer Trainium2 kernel (raw Bass, no Tile).

Math: the reference's sort/std/rank machinery is dead code (bit_token is
unconditionally overwritten with n_bits), so the computation reduces to:
    delta[b,t] = max over (h,c) of x[b,h,t,c]
    out = delta * 2^(round(log2(max(x/delta, 1e-8))))
i.e. snap x/delta to the nearest power of two in log space, rescale by delta.

Division-route bit-trick (no transcendentals), exact on the fp32-internal DVE:
    q  = x * (1 / (delta*sqrt2))             (reciprocal is IEEE 1/x on trn2)
    p2 = bitcast_f32(bits(q) & 0x7F800000)   # 2^floor(log2 q) = 2^(k-1)
    out = p2 * (2*delta)                     # fp32 mult by 2^k, exact
round(log2(x/delta)) = floor(log2(x/(delta*sqrt2))) + 1, so flooring q to its
exponent implements the rounding; x==0 gives q=0 -> p2=+0.0 -> out=0 (the
reference's 1e-8 ratio clamp yields delta*2^-27 ~ 7e-9 there; abs err 7e-9).

Sharding: data-parallel over batch dim b (8 rows -> 8 cores), no comms.
Layout: t split into chunks; partition dim = t-block so each partition line is
one contiguous run per h in DRAM. Per-token scalars broadcast along the free
(h, c) dims with stride-0 APs.

Engine split and pipeline (NBUF-deep buffers):
  DVE:    R1+R2 reduces, recip smalls, M1 mult, AND mask (~9us/chunk)
  GpSimd: M2 final fp32 mult (tensor_tensor)             (~5.3us/chunk)
  Sync:   HWDGE DMAs
Per-chunk chain: load -> R1,R2,smalls,M1,AND (dve) -> M2 (gp) -> store
Same-engine ops execute in order on trn2 (per-op pipeline drain), so only
cross-engine edges carry semaphores:
  dve_sem: +1 by AND (last DVE op of a chunk); gp waits it before M2
  v_sem:   +1 by M2 (gp); sync waits it before the store
  load_sem/store_sem[NBUF]: buffer-slot DMA completion sems (16/DMA); each
    slot's counter is only ever one-DMA-in-flight, so thresholds are exact
"""

from contextlib import ExitStack

import numpy as np

import concourse.bass as bass
import concourse.mybir as mybir
from concourse.bass_utils import run_bass_kernel_spmd

B, H, T, C = 8, 12, 4096, 64
N_CORES = 8
P = 128          # SBUF partitions
TC = 512         # tokens per chunk (pipeline granularity)
NBUF = 4         # xt/wt buffer depth

SQRT2 = 1.4142135623730951
EXP_MASK = 0x7F800000

_nc_cache = {}


def _build_nc():
    if "nc" in _nc_cache:
        return _nc_cache["nc"]
    f32 = mybir.dt.float32
    i32 = mybir.dt.int32
    OP = mybir.AluOpType
    X = mybir.AxisListType.X

    nc = bass.Bass()
    x_in = nc.declare_dram_parameter("x", [H, T, C], f32, isOutput=False)
    y_out = nc.declare_dram_parameter("y", [H, T, C], f32, isOutput=True)

    n_chunks = T // TC
    tt = TC // P
    FREE = H * tt * C

    def src_ap(ci):
        return x_in[:, ci * TC : (ci + 1) * TC, :].rearrange(
            "h (p q) c -> p h (q c)", p=P
        )

    def dst_ap(ci):
        return y_out[:, ci * TC : (ci + 1) * TC, :].rearrange(
            "h (p q) c -> p h (q c)", p=P
        )

    with ExitStack() as ctx:
        xt = [
            ctx.enter_context(nc.sbuf_tensor(f"xt{j}", [P, FREE], f32))
            for j in range(NBUF)
        ]
        wt = [
            ctx.enter_context(nc.sbuf_tensor(f"wt{j}", [P, FREE], f32))
            for j in range(NBUF)
        ]
        r1 = ctx.enter_context(nc.sbuf_tensor("r1", [P, H * tt], f32))
        delta = ctx.enter_context(nc.sbuf_tensor("delta", [P, tt], f32))
        tf = ctx.enter_context(nc.sbuf_tensor("tf", [P, tt], f32))
        inv = ctx.enter_context(nc.sbuf_tensor("inv", [P, tt], f32))
        # d2 is read cross-engine by gp's M2 -> per-slot copies so the next
        # DVE chunks never wait for M2
        d2 = [
            ctx.enter_context(nc.sbuf_tensor(f"d2_{j}", [P, tt], f32))
            for j in range(NBUF)
        ]

        load_sem = [
            ctx.enter_context(nc.semaphore(f"load_sem{j}")) for j in range(NBUF)
        ]
        store_sem = [
            ctx.enter_context(nc.semaphore(f"store_sem{j}")) for j in range(NBUF)
        ]
        v_sem = ctx.enter_context(nc.semaphore("v_sem"))
        dve_sem = ctx.enter_context(nc.semaphore("dve_sem"))

        block = ctx.enter_context(nc.Block())

        @block.sync
        def _(sync):
            for ci in range(n_chunks):
                sync.dma_start(out=xt[ci % NBUF][:], in_=src_ap(ci)).then_inc(
                    load_sem[ci % NBUF], 16
                )
                if ci >= 1:
                    sync.wait_ge(v_sem, ci)
                    sync.dma_start(
                        out=dst_ap(ci - 1), in_=wt[(ci - 1) % NBUF][:]
                    ).then_inc(store_sem[(ci - 1) % NBUF], 16)
            sync.wait_ge(v_sem, n_chunks)
            sync.dma_start(
                out=dst_ap(n_chunks - 1), in_=wt[(n_chunks - 1) % NBUF][:]
            ).then_inc(store_sem[(n_chunks - 1) % NBUF], 16)

        @block.gpsimd
        def _(gp):
            for ci in range(n_chunks):
                j = ci % NBUF
                xt4 = xt[j][:].rearrange("p (h q c) -> p h q c", h=H, c=C)
                wt4 = wt[j][:].rearrange("p (h q c) -> p h q c", h=H, c=C)

                # M2: out = p2 * 2*delta  (xt -> wt; wt dead after AND)
                d2_b = d2[j][:].unsqueeze(1).unsqueeze(3).broadcast_to([P, H, tt, C])
                gp.wait_ge(dve_sem, 7 * (ci + 1))               # AND(ci) done
                gp.tensor_tensor(out=wt4, in0=xt4, in1=d2_b, op=OP.mult).then_inc(
                    v_sem, 1
                )

        @block.vector
        def _(vector):
            for ci in range(n_chunks):
                j = ci % NBUF
                xt4 = xt[j][:].rearrange("p (h q c) -> p h q c", h=H, c=C)
                wt4 = wt[j][:].rearrange("p (h q c) -> p h q c", h=H, c=C)
                r13 = r1[:].rearrange("p (h q) -> p h q", h=H)

                if ci >= NBUF:
                    vector.wait_ge(store_sem[j], 16 * (ci // NBUF))  # wt free
                vector.wait_ge(load_sem[j], 16 * (ci // NBUF + 1))   # xt loaded

                # DVE ops overlap in the pipe; dependent pairs are ordered by
                # counting-sem fences (DVE ops inc dve_sem, consumers wait).
                # 7 incs/chunk: R1=1 R2=2 tf=3 inv=4 d2=5 M1=6 AND=7.
                b = 7 * ci
                # R1: per-(token,h) max over c; R2: delta = max over h
                vector.reduce_max(out=r13, in_=xt4, axis=X).then_inc(dve_sem, 1)
                vector.wait_ge(dve_sem, b + 1)
                vector.reduce_max(
                    out=delta[:], in_=r13.transpose([0, 2, 1]), axis=X
                ).then_inc(dve_sem, 1)
                vector.wait_ge(dve_sem, b + 2)
                # per-token scalars
                vector.tensor_scalar_mul(tf[:], delta[:], SQRT2).then_inc(dve_sem, 1)
                vector.wait_ge(dve_sem, b + 3)
                vector.reciprocal(inv[:], tf[:]).then_inc(dve_sem, 1)
                vector.tensor_scalar_mul(d2[j][:], delta[:], 2.0).then_inc(dve_sem, 1)

                inv_b = inv[:].unsqueeze(1).unsqueeze(3).broadcast_to([P, H, tt, C])

                # M1: q = x * inv
                vector.wait_ge(dve_sem, b + 4)
                vector.tensor_tensor(out=wt4, in0=xt4, in1=inv_b, op=OP.mult).then_inc(
                    dve_sem, 1
                )
                # AND: p2 = bits(q) & 0x7F800000  (wt -> xt, xt dead after M1)
                vector.wait_ge(dve_sem, b + 6)
                vector.tensor_scalar(
                    out=xt[j][:].bitcast(i32),
                    in0=wt[j][:].bitcast(i32),
                    scalar1=EXP_MASK,
                    scalar2=None,
                    op0=OP.bitwise_and,
                ).then_inc(dve_sem, 1)

    _nc_cache["nc"] = nc
    return nc


def kernel(x: np.ndarray) -> np.ndarray:
    assert x.shape == (B, H, T, C) and x.dtype == np.float32
    nc = _build_nc()
    in_maps = [{"x": np.ascontiguousarray(x[i])} for i in range(N_CORES)]
    res = run_bass_kernel_spmd(nc, in_maps, list(range(N_CORES)))
    out = np.stack([res.results[i]["y"] for i in range(N_CORES)], axis=0)
    return out


# revision 27
# speedup vs baseline: 1.3483x; 1.2681x over previous
"""Log2Quantizer Trainium2 kernel (raw Bass, no Tile).

Math: the reference's sort/std/rank machinery is dead code (bit_token is
unconditionally overwritten with n_bits), so the computation reduces to:
    delta[b,t] = max over (h,c) of x[b,h,t,c]
    out = delta * 2^(round(log2(max(x/delta, 1e-8))))
i.e. snap x/delta to the nearest power of two in log space, rescale by delta.

Division-route bit-trick (no transcendentals), exact on the fp32-internal DVE:
    q  = x * (1 / (delta*sqrt2))             (reciprocal is IEEE 1/x on trn2)
    p2 = bitcast_f32(bits(q) & 0x7F800000)   # 2^floor(log2 q) = 2^(k-1)
    out = p2 * (2*delta)                     # fp32 mult by 2^k, exact
round(log2(x/delta)) = floor(log2(x/(delta*sqrt2))) + 1, so flooring q to its
exponent implements the rounding; x==0 gives q=0 -> p2=+0.0 -> out=0 (the
reference's 1e-8 ratio clamp yields delta*2^-27 ~ 7e-9 there; abs err 7e-9).

Sharding: data-parallel over batch dim b (8 rows -> 8 cores), no comms.
Layout: t split into TC=512-token chunks; partition dim = t-block of 4 so each
partition line is one contiguous 1KB run per h in DRAM (fast DMA). Compute
sub-steps each chunk into 4 x 128-token slices where per-token scalars are
per-partition [128,1] APs -> tensor_scalar runs in the DVE's 2x port mode.

Engines:
  Sync (SP HWDGE ring):    loads
  Scalar (ACT HWDGE ring): stores  (separate FIFO so loads never queue
                           behind stores; ACT is otherwise idle)
  DVE:  R1+R2 reduces, recip smalls, M1 (4 sub-slices, 2x), AND (2x)
  GpSimd: M2 final fp32 mult (tensor_tensor)
Cross-engine sems (one update per instruction):
  dve_sem: +1 per DVE op (10/chunk: R1,R2,tf,recip,d2,M1x4,AND);
           fences order the same-engine dependent pairs (DVE ops overlap
           in the pipe without them - verified corrupting on HW)
  v_sem:   +1 by M2 (gp); stores wait it; loads wait it NBUF chunks back
  load_sem/store_sem[NBUF]: per-slot DMA completion (16/DMA)
"""

from contextlib import ExitStack

import numpy as np

import concourse.bass as bass
import concourse.mybir as mybir
from concourse.bass_utils import run_bass_kernel_spmd

B, H, T, C = 8, 12, 4096, 64
N_CORES = 8
P = 128          # SBUF partitions
TC = 512         # tokens per chunk (pipeline granularity)
NBUF = 4         # xt/wt buffer depth

SQRT2 = 1.4142135623730951
EXP_MASK = 0x7F800000
DVE_INCS = 10

_nc_cache = {}


def _build_nc():
    if "nc" in _nc_cache:
        return _nc_cache["nc"]
    f32 = mybir.dt.float32
    i32 = mybir.dt.int32
    OP = mybir.AluOpType
    X = mybir.AxisListType.X

    nc = bass.Bass()
    x_in = nc.declare_dram_parameter("x", [H, T, C], f32, isOutput=False)
    y_out = nc.declare_dram_parameter("y", [H, T, C], f32, isOutput=True)

    n_chunks = T // TC
    tt = TC // P
    FREE = H * tt * C

    def src_ap(ci):
        return x_in[:, ci * TC : (ci + 1) * TC, :].rearrange(
            "h (p q) c -> p h (q c)", p=P
        )

    def dst_ap(ci):
        return y_out[:, ci * TC : (ci + 1) * TC, :].rearrange(
            "h (p q) c -> p h (q c)", p=P
        )

    with ExitStack() as ctx:
        xt = [
            ctx.enter_context(nc.sbuf_tensor(f"xt{j}", [P, FREE], f32))
            for j in range(NBUF)
        ]
        wt = [
            ctx.enter_context(nc.sbuf_tensor(f"wt{j}", [P, FREE], f32))
            for j in range(NBUF)
        ]
        r1 = ctx.enter_context(nc.sbuf_tensor("r1", [P, H * tt], f32))
        delta = ctx.enter_context(nc.sbuf_tensor("delta", [P, tt], f32))
        tf = ctx.enter_context(nc.sbuf_tensor("tf", [P, tt], f32))
        inv = ctx.enter_context(nc.sbuf_tensor("inv", [P, tt], f32))
        # d2 is read cross-engine by gp's M2 -> per-slot copies so DVE never
        # waits for M2 before overwriting
        d2 = [
            ctx.enter_context(nc.sbuf_tensor(f"d2_{j}", [P, tt], f32))
            for j in range(NBUF)
        ]

        load_sem = [
            ctx.enter_context(nc.semaphore(f"load_sem{j}")) for j in range(NBUF)
        ]
        store_sem = [
            ctx.enter_context(nc.semaphore(f"store_sem{j}")) for j in range(NBUF)
        ]
        v_sem = ctx.enter_context(nc.semaphore("v_sem"))
        dve_sem = ctx.enter_context(nc.semaphore("dve_sem"))

        block = ctx.enter_context(nc.Block())

        @block.sync
        def _(sync):
            # loads only; SP HWDGE ring
            for ci in range(n_chunks):
                if ci >= NBUF:
                    # xt slot's last reader is M2(ci-NBUF) on gp
                    sync.wait_ge(v_sem, ci - NBUF + 1)
                sync.dma_start(out=xt[ci % NBUF][:], in_=src_ap(ci)).then_inc(
                    load_sem[ci % NBUF], 16
                )

        @block.scalar
        def _(scalar):
            # stores only; ACT HWDGE ring (independent FIFO from loads)
            for ci in range(n_chunks):
                scalar.wait_ge(v_sem, ci + 1)
                scalar.dma_start(out=dst_ap(ci), in_=wt[ci % NBUF][:]).then_inc(
                    store_sem[ci % NBUF], 16
                )

        @block.gpsimd
        def _(gp):
            for ci in range(n_chunks):
                j = ci % NBUF
                xt4 = xt[j][:].rearrange("p (h q c) -> p h q c", h=H, c=C)
                wt4 = wt[j][:].rearrange("p (h q c) -> p h q c", h=H, c=C)

                # M2: out = p2 * 2*delta  (xt -> wt; wt dead after AND)
                d2_b = d2[j][:].unsqueeze(1).unsqueeze(3).broadcast_to([P, H, tt, C])
                gp.wait_ge(dve_sem, DVE_INCS * (ci + 1))        # AND(ci) done
                gp.tensor_tensor(out=wt4, in0=xt4, in1=d2_b, op=OP.mult).then_inc(
                    v_sem, 1
                )

        @block.vector
        def _(vector):
            for ci in range(n_chunks):
                j = ci % NBUF
                xt4 = xt[j][:].rearrange("p (h q c) -> p h q c", h=H, c=C)
                wt4 = wt[j][:].rearrange("p (h q c) -> p h q c", h=H, c=C)
                r13 = r1[:].rearrange("p (h q) -> p h q", h=H)

                if ci >= NBUF:
                    vector.wait_ge(store_sem[j], 16 * (ci // NBUF))  # wt free
                vector.wait_ge(load_sem[j], 16 * (ci // NBUF + 1))   # xt loaded

                b = DVE_INCS * ci
                # R1: per-(token,h) max over c; R2: delta = max over h
                vector.reduce_max(out=r13, in_=xt4, axis=X).then_inc(dve_sem, 1)
                vector.wait_ge(dve_sem, b + 1)
                vector.reduce_max(
                    out=delta[:], in_=r13.transpose([0, 2, 1]), axis=X
                ).then_inc(dve_sem, 1)
                vector.wait_ge(dve_sem, b + 2)
                # per-token scalars
                vector.tensor_scalar_mul(tf[:], delta[:], SQRT2).then_inc(dve_sem, 1)
                vector.wait_ge(dve_sem, b + 3)
                vector.reciprocal(inv[:], tf[:]).then_inc(dve_sem, 1)
                vector.tensor_scalar_mul(d2[j][:], delta[:], 2.0).then_inc(dve_sem, 1)

                # M1: q = x * inv, sub-stepped so inv slice is [128,1]
                # (per-partition scalar -> DVE 2x port mode)
                vector.wait_ge(dve_sem, b + 4)                   # recip done
                for s in range(tt):
                    vector.tensor_scalar_mul(
                        wt4[:, :, s, :], xt4[:, :, s, :], inv[:, s : s + 1]
                    ).then_inc(dve_sem, 1)
                # AND: p2 = bits(q) & 0x7F800000  (wt -> xt, xt dead after M1)
                vector.wait_ge(dve_sem, b + 5 + tt)              # all M1 done
                vector.tensor_scalar(
                    out=xt[j][:].bitcast(i32),
                    in0=wt[j][:].bitcast(i32),
                    scalar1=EXP_MASK,
                    scalar2=None,
                    op0=OP.bitwise_and,
                ).then_inc(dve_sem, 1)

    _nc_cache["nc"] = nc
    return nc


def kernel(x: np.ndarray) -> np.ndarray:
    assert x.shape == (B, H, T, C) and x.dtype == np.float32
    nc = _build_nc()
    in_maps = [{"x": np.ascontiguousarray(x[i])} for i in range(N_CORES)]
    res = run_bass_kernel_spmd(nc, in_maps, list(range(N_CORES)))
    out = np.stack([res.results[i]["y"] for i in range(N_CORES)], axis=0)
    return out


# revision 30
# speedup vs baseline: 1.4077x; 1.0441x over previous
"""Log2Quantizer Trainium2 kernel (raw Bass, no Tile).

Math: the reference's sort/std/rank machinery is dead code (bit_token is
unconditionally overwritten with n_bits), so the computation reduces to:
    delta[b,t] = max over (h,c) of x[b,h,t,c]
    out = delta * 2^(round(log2(max(x/delta, 1e-8))))
i.e. snap x/delta to the nearest power of two in log space, rescale by delta.

Division-route bit-trick (no transcendentals), exact on the fp32-internal DVE:
    q  = x * (1 / (delta*sqrt2))             (reciprocal is IEEE 1/x on trn2)
    p2 = bitcast_f32(bits(q) & 0x7F800000)   # 2^floor(log2 q) = 2^(k-1)
    out = p2 * (2*delta)                     # fp32 mult by 2^k, exact
round(log2(x/delta)) = floor(log2(x/(delta*sqrt2))) + 1, so flooring q to its
exponent implements the rounding; x==0 gives q=0 -> p2=+0.0 -> out=0 (the
reference's 1e-8 ratio clamp yields delta*2^-27 ~ 7e-9 there; abs err 7e-9).

Sharding: data-parallel over batch dim b (8 rows -> 8 cores), no comms.
Layout: t split into TC=512-token chunks; partition dim = t-block of 4 so each
partition line is one contiguous 1KB run per h in DRAM (fast DMA). Compute
sub-steps each chunk into 4 x 128-token slices where per-token scalars are
per-partition [128,1] APs -> tensor_scalar runs in the DVE's 2x port mode.

Engines:
  Sync (SP HWDGE ring):    loads
  Scalar (ACT HWDGE ring): stores  (separate FIFO so loads never queue
                           behind stores; ACT is otherwise idle)
  DVE:  R1+R2 reduces, recip smalls, M1 (4 sub-slices, 2x), AND (2x)
  GpSimd: M2 final fp32 mult (tensor_tensor)
Cross-engine sems (one update per instruction):
  dve_sem: +1 per DVE op (10/chunk: R1,R2,tf,recip,d2,M1x4,AND);
           fences order the same-engine dependent pairs (DVE ops overlap
           in the pipe without them - verified corrupting on HW)
  v_sem:   +1 by M2 (gp); stores wait it; loads wait it NBUF chunks back
  load_sem/store_sem[NBUF]: per-slot DMA completion (16/DMA)
"""

from contextlib import ExitStack

import numpy as np

import concourse.bass as bass
import concourse.mybir as mybir
from concourse.bass_utils import run_bass_kernel_spmd

B, H, T, C = 8, 12, 4096, 64
N_CORES = 8
P = 128          # SBUF partitions
TC = 512         # tokens per chunk (pipeline granularity)
NBUF = 4         # xt/wt buffer depth

ISQRT2 = 0.7071067811865476
EXP_MASK = 0x7F800000
DVE_INCS = 9

_nc_cache = {}


def _build_nc():
    if "nc" in _nc_cache:
        return _nc_cache["nc"]
    f32 = mybir.dt.float32
    i32 = mybir.dt.int32
    OP = mybir.AluOpType
    X = mybir.AxisListType.X

    nc = bass.Bass()
    x_in = nc.declare_dram_parameter("x", [H, T, C], f32, isOutput=False)
    y_out = nc.declare_dram_parameter("y", [H, T, C], f32, isOutput=True)

    n_chunks = T // TC
    tt = TC // P
    FREE = H * tt * C

    def src_ap(ci):
        return x_in[:, ci * TC : (ci + 1) * TC, :].rearrange(
            "h (p q) c -> p h (q c)", p=P
        )

    def dst_ap(ci):
        return y_out[:, ci * TC : (ci + 1) * TC, :].rearrange(
            "h (p q) c -> p h (q c)", p=P
        )

    with ExitStack() as ctx:
        xt = [
            ctx.enter_context(nc.sbuf_tensor(f"xt{j}", [P, FREE], f32))
            for j in range(NBUF)
        ]
        wt = [
            ctx.enter_context(nc.sbuf_tensor(f"wt{j}", [P, FREE], f32))
            for j in range(NBUF)
        ]
        r1 = ctx.enter_context(nc.sbuf_tensor("r1", [P, H * tt], f32))
        delta = ctx.enter_context(nc.sbuf_tensor("delta", [P, tt], f32))
        inv = ctx.enter_context(nc.sbuf_tensor("inv", [P, tt], f32))
        # d2 is read cross-engine by gp's M2 -> per-slot copies so DVE never
        # waits for M2 before overwriting
        d2 = [
            ctx.enter_context(nc.sbuf_tensor(f"d2_{j}", [P, tt], f32))
            for j in range(NBUF)
        ]

        load_sem = [
            ctx.enter_context(nc.semaphore(f"load_sem{j}")) for j in range(NBUF)
        ]
        store_sem = [
            ctx.enter_context(nc.semaphore(f"store_sem{j}")) for j in range(NBUF)
        ]
        v_sem = ctx.enter_context(nc.semaphore("v_sem"))
        dve_sem = ctx.enter_context(nc.semaphore("dve_sem"))

        block = ctx.enter_context(nc.Block())

        @block.sync
        def _(sync):
            # loads only; SP HWDGE ring
            for ci in range(n_chunks):
                if ci >= NBUF:
                    # xt slot's last reader is M2(ci-NBUF) on gp
                    sync.wait_ge(v_sem, ci - NBUF + 1)
                sync.dma_start(out=xt[ci % NBUF][:], in_=src_ap(ci)).then_inc(
                    load_sem[ci % NBUF], 16
                )

        @block.scalar
        def _(scalar):
            # stores only; ACT HWDGE ring (independent FIFO from loads)
            for ci in range(n_chunks):
                scalar.wait_ge(v_sem, ci + 1)
                scalar.dma_start(out=dst_ap(ci), in_=wt[ci % NBUF][:]).then_inc(
                    store_sem[ci % NBUF], 16
                )

        @block.gpsimd
        def _(gp):
            for ci in range(n_chunks):
                j = ci % NBUF
                xt4 = xt[j][:].rearrange("p (h q c) -> p h q c", h=H, c=C)
                wt4 = wt[j][:].rearrange("p (h q c) -> p h q c", h=H, c=C)

                # M2: out = p2 * 2*delta  (xt -> wt; wt dead after AND)
                d2_b = d2[j][:].unsqueeze(1).unsqueeze(3).broadcast_to([P, H, tt, C])
                gp.wait_ge(dve_sem, DVE_INCS * (ci + 1))        # AND(ci) done
                gp.tensor_tensor(out=wt4, in0=xt4, in1=d2_b, op=OP.mult).then_inc(
                    v_sem, 1
                )

        @block.vector
        def _(vector):
            for ci in range(n_chunks):
                j = ci % NBUF
                xt4 = xt[j][:].rearrange("p (h q c) -> p h q c", h=H, c=C)
                wt4 = wt[j][:].rearrange("p (h q c) -> p h q c", h=H, c=C)
                r13 = r1[:].rearrange("p (h q) -> p h q", h=H)

                if ci >= NBUF:
                    vector.wait_ge(store_sem[j], 16 * (ci // NBUF))  # wt free
                vector.wait_ge(load_sem[j], 16 * (ci // NBUF + 1))   # xt loaded

                b = DVE_INCS * ci
                # R1: per-(token,h) max over c; R2: delta = max over h
                vector.reduce_max(out=r13, in_=xt4, axis=X).then_inc(dve_sem, 1)
                vector.wait_ge(dve_sem, b + 1)
                vector.reduce_max(
                    out=delta[:], in_=r13.transpose([0, 2, 1]), axis=X
                ).then_inc(dve_sem, 1)
                vector.wait_ge(dve_sem, b + 2)
                # per-token scalars: inv = 1/delta, d2 = 2*delta
                vector.reciprocal(inv[:], delta[:]).then_inc(dve_sem, 1)
                vector.tensor_scalar_mul(d2[j][:], delta[:], 2.0).then_inc(dve_sem, 1)

                # M1: q = (x * inv) * (1/sqrt2), sub-stepped so the inv slice
                # is a [128,1] per-partition scalar -> DVE 2x port mode;
                # 1/sqrt2 rides the second scalar-op slot
                vector.wait_ge(dve_sem, b + 3)                   # recip done
                for s in range(tt):
                    vector.tensor_scalar(
                        out=wt4[:, :, s, :],
                        in0=xt4[:, :, s, :],
                        scalar1=inv[:, s : s + 1],
                        scalar2=ISQRT2,
                        op0=OP.mult,
                        op1=OP.mult,
                    ).then_inc(dve_sem, 1)
                # AND: p2 = bits(q) & 0x7F800000  (wt -> xt, xt dead after M1)
                vector.wait_ge(dve_sem, b + 4 + tt)              # all M1 done
                vector.tensor_scalar(
                    out=xt[j][:].bitcast(i32),
                    in0=wt[j][:].bitcast(i32),
                    scalar1=EXP_MASK,
                    scalar2=None,
                    op0=OP.bitwise_and,
                ).then_inc(dve_sem, 1)

    _nc_cache["nc"] = nc
    return nc


def kernel(x: np.ndarray) -> np.ndarray:
    assert x.shape == (B, H, T, C) and x.dtype == np.float32
    nc = _build_nc()
    in_maps = [{"x": np.ascontiguousarray(x[i])} for i in range(N_CORES)]
    res = run_bass_kernel_spmd(nc, in_maps, list(range(N_CORES)))
    out = np.stack([res.results[i]["y"] for i in range(N_CORES)], axis=0)
    return out
